# revision 27
# baseline (speedup 1.0000x reference)
"""Multihead attention (B=4, S=2048, E=1024, H=16, D=64) on 8 Trainium2 cores.

Sharding: core c = (batch b = c//2, head-half hh = c%2). Each core computes one
batch's attention for 8 heads (512 of the 1024 projection columns), producing a
partial output (row-split Wo); the host sums the two partials per batch.

v2 design (vs the 775us v1):
- Host pre-transposes x (xT [E,S] f16) so Phase A is pure projection matmuls
  (no PE transposes, no DVE transpose copies). All DMA'd operands are f16.
- Score matmuls for the two heads of a pair use PE row-tiling (K=64 at
  partitions 0-63 / 64-127) and run concurrently.
- exp() is split between ScalarE (true exp) and VectorE (Schraudolph bit-trick
  exp: one affine f32->int16 op whose result bitcast as f16 approximates
  exp to ~3%) so the 33M-element softmax doesn't serialize behind ScalarE.
- Phase C (output projection) is interleaved per sq-chunk with Phase B to
  keep the PE dense; output DMA'd as f16, host does the final cast/sum.
"""
import sys

sys.path.insert(0, "/opt/trn_rl_repo")

import numpy as np

import concourse.bacc as bacc
import concourse.mybir as mybir
import concourse.tile as tile
from concourse.bass_utils import run_bass_kernel_spmd

E = 1024
H = 16
D = 64
B = 4
S = 2048
HH = E // 2          # projection cols per core
N_CORES = 8
P = 128
NCH = 4              # sq-chunks of 512
CH = 512
f32 = mybir.dt.float32
f16 = mybir.dt.float16
i16 = mybir.dt.int16
AF = mybir.ActivationFunctionType
ALU = mybir.AluOpType

# Schraudolph fast exp on DVE: exp(s*0.125) ~= bitcast_f16(int16(s*SCH_A + SCH_B))
SCH_A = 0.125 * 1024.0 / float(np.log(2.0))   # 184.664
SCH_B = 15360.0 - 44.0
# fp32 bit-trick reciprocal seed (refined by one Newton step on DVE)
RMAGIC = 0x7EF311C3

_cached = {}


def _build():
    nc = bacc.Bacc(None, target_bir_lowering=False)

    xqT = nc.declare_dram_parameter("xqT", [E, S], f16, isOutput=False)
    xkT = nc.declare_dram_parameter("xkT", [E, S], f16, isOutput=False)
    xvT = nc.declare_dram_parameter("xvT", [E, S], f16, isOutput=False)
    wq = nc.declare_dram_parameter("wq", [P, 8, HH], f16, isOutput=False)
    wk = nc.declare_dram_parameter("wk", [P, 8, HH], f16, isOutput=False)
    wv = nc.declare_dram_parameter("wv", [P, 8, HH], f16, isOutput=False)
    bq_col = nc.declare_dram_parameter("bq_col", [P, 4], f32, isOutput=False)
    bk_col = nc.declare_dram_parameter("bk_col", [P, 4], f32, isOutput=False)
    bv_bc = nc.declare_dram_parameter("bv_bc", [P, 8, D], f16, isOutput=False)
    wo = nc.declare_dram_parameter("wo", [P, 4, E], f16, isOutput=False)
    bo_col = nc.declare_dram_parameter("bo_col", [P, 8], f32, isOutput=False)
    yT = nc.declare_dram_parameter("yT", [E, S], f16, isOutput=True)

    from contextlib import ExitStack

    with tile.TileContext(nc) as tc, ExitStack() as stack:
        main = stack.enter_context(tc.tile_pool(name="main", bufs=1))
        qT = main.tile([P, 4, S], f16)      # [d-in-pair, pair, sq]
        kT = main.tile([P, 4, S], f16)
        vbuf = main.tile([P, 16, 8, D + 1], f16)  # [sv, s-tile, head, d|1]
        ou = main.tile([P, 4, S], f16)      # attention out (normalized), [d-in-pair, pair, sq]
        wo_t = main.tile([P, 4, E], f16)
        bqc = main.tile([P, 4], f32)
        bkc = main.tile([P, 4], f32)
        boc = main.tile([P, 8], f32)
        bvt = main.tile([P, 8, D], f16)
        pones = main.tile([P, 64], f16)

        nc.vector.memset(pones[:], 1.0)
        nc.vector.memset(vbuf[:, :, :, D], 1.0)

        nc.sync.dma_start(out=bqc[:], in_=bq_col[:])
        nc.sync.dma_start(out=bkc[:], in_=bk_col[:])
        nc.sync.dma_start(out=bvt[:], in_=bv_bc[:])

        # ---------------- Phase A: projections (x comes in pre-transposed) ----
        with tc.tile_pool(name="wp", bufs=2) as wp, \
             tc.tile_pool(name="xp", bufs=2) as xp, \
             tc.tile_pool(name="ps_pj", bufs=4, space="PSUM") as ps_pj:
            for xdram, wdram, kind in ((xkT, wk, "k"), (xqT, wq, "q"), (xvT, wv, "v")):
                w_t = wp.tile([P, 8, HH], f16, tag="w", name=f"w_{kind}")
                nc.sync.dma_start(out=w_t[:], in_=wdram[:])
                x_t = xp.tile([P, 8, S], f16, tag="x", name=f"x_{kind}")
                for kc in range(8):
                    nc.gpsimd.dma_start(out=x_t[:, kc, :],
                                        in_=xdram[kc * P:(kc + 1) * P, :])
                if kind == "v":
                    for sv in range(16):
                        pp = ps_pj.tile([P, 8, D], f32, tag="pj", name=f"pj_v{sv}")
                        for kc in range(8):
                            nc.tensor.matmul(pp[:], lhsT=x_t[:, kc, sv * P:(sv + 1) * P],
                                             rhs=w_t[:, kc, :],
                                             start=(kc == 0), stop=(kc == 7))
                        nc.vector.tensor_add(vbuf[:, sv, :, 0:D], pp[:], bvt[:])
                else:
                    dest = qT if kind == "q" else kT
                    bcol = bqc if kind == "q" else bkc
                    for u in range(4):
                        for g in range(4):
                            pp = ps_pj.tile([P, CH], f32, tag="pj",
                                            name=f"pj_{kind}{u}{g}")
                            for kc in range(8):
                                nc.tensor.matmul(pp[:], lhsT=w_t[:, kc, u * P:(u + 1) * P],
                                                 rhs=x_t[:, kc, g * CH:(g + 1) * CH],
                                                 start=(kc == 0), stop=(kc == 7))
                            nc.scalar.add(dest[:, u, g * CH:(g + 1) * CH],
                                          pp[:], bcol[:, u:u + 1])

        nc.sync.dma_start(out=boc[:], in_=bo_col[:])
        nc.sync.dma_start(out=wo_t[:], in_=wo[:])

        # ---------------- Phase B: attention, Phase C: out-proj (interleaved per c)
        with tc.tile_pool(name="ep", bufs=1) as ep, \
             tc.tile_pool(name="ivp", bufs=2) as ivp, \
             tc.tile_pool(name="otp", bufs=3) as otp, \
             tc.tile_pool(name="ps_a", bufs=2, space="PSUM") as ps_a, \
             tc.tile_pool(name="ps_b", bufs=2, space="PSUM") as ps_b, \
             tc.tile_pool(name="ps_ac", bufs=2, space="PSUM") as ps_ac:
            norm_pend = []

            def flush_norm():
                while norm_pend:
                    fpso, fbc, fpr, fcs = norm_pend.pop(0)
                    nc.vector.tensor_mul(ou[0:64, fpr, fcs], fpso[0:64, 0, :],
                                         fbc[:, 0, :])
                    nc.vector.tensor_mul(ou[64:128, fpr, fcs], fpso[0:64, 1, :],
                                         fbc[:, 1, :])

            for c in range(NCH):
                cs = slice(c * CH, (c + 1) * CH)
                for pr in range(4):
                    hA, hB = 2 * pr, 2 * pr + 1
                    # A and B halves share one 2-bank accumulator so the two
                    # softmax denominators form a single [1, 1024] row
                    pso = ps_ac.tile([D + 1, 2, CH], f32, tag="p", bufs=2)
                    pend = []
                    for st in range(16):
                        ks = slice(st * P, (st + 1) * P)
                        pscA = ps_a.tile([P, CH], f32, tag="a", bufs=2)
                        pscB = ps_b.tile([P, CH], f32, tag="b", bufs=2)
                        # two concurrent K=64 row-tiled score matmuls
                        nc.tensor.matmul(pscA[:], lhsT=kT[0:64, pr, ks],
                                         rhs=qT[0:64, pr, cs], start=True, stop=True)
                        nc.tensor.matmul(pscB[:], lhsT=kT[64:128, pr, ks],
                                         rhs=qT[64:128, pr, cs], start=True, stop=True)
                        exA = ep.tile([P, CH], f16, tag="xa", bufs=6)
                        exB = ep.tile([P, CH], f16, tag="xb", bufs=6)
                        # exp split (~20 ScalarE / 12 DVE per chunk): ScalarE
                        # true exp, DVE Schraudolph bit-trick exp
                        nc.scalar.activation(exA[:], pscA[:], AF.Exp, scale=0.125)
                        if st % 4 == 0:
                            nc.scalar.activation(exB[:], pscB[:], AF.Exp, scale=0.125)
                        else:
                            nc.vector.tensor_scalar(out=exB[:].bitcast(i16),
                                                    in0=pscB[:], scalar1=SCH_A,
                                                    scalar2=SCH_B,
                                                    op0=ALU.mult, op1=ALU.add)
                        # attnV runs two steps behind scores on the PE queue:
                        # by the time the PE reaches attnV(st-2), exp(st-2) is
                        # long done, so the PE never stalls mid-chunk (stalls
                        # break the HAM activity window and halve the clock)
                        if st == 2:
                            flush_norm()
                        pend.append((st, exA, exB))
                        if len(pend) > 2:
                            pst, pA, pB = pend.pop(0)
                            nc.tensor.matmul(pso[:, 0, :], lhsT=vbuf[:, pst, hA, :],
                                             rhs=pA[:], start=(pst == 0), stop=False,
                                             skip_group_check=True)
                            nc.tensor.matmul(pso[:, 1, :], lhsT=vbuf[:, pst, hB, :],
                                             rhs=pB[:], start=(pst == 0), stop=False,
                                             skip_group_check=True)
                    for pst, pA, pB in pend:
                        nc.tensor.matmul(pso[:, 0, :], lhsT=vbuf[:, pst, hA, :],
                                         rhs=pA[:], start=False, stop=(pst == 15),
                                         skip_group_check=True)
                        nc.tensor.matmul(pso[:, 1, :], lhsT=vbuf[:, pst, hB, :],
                                         rhs=pB[:], start=False, stop=(pst == 15),
                                         skip_group_check=True)
                    # -1/den: bit-trick seed + one Newton step on DVE (sign
                    # folded into Wo on the host)
                    sd = ivp.tile([1, 2, CH], f32, tag="sd", bufs=2)
                    tt = ivp.tile([1, 2, CH], f32, tag="tt", bufs=2)
                    inv = ivp.tile([1, 2, CH], f16, tag="iv", bufs=2)
                    nc.vector.tensor_scalar(out=sd[0:1].bitcast(mybir.dt.int32),
                                            in0=pso[64:65, :, :].bitcast(mybir.dt.int32),
                                            scalar1=-1, scalar2=RMAGIC,
                                            op0=ALU.mult, op1=ALU.add)
                    nc.vector.tensor_mul(tt[0:1], sd[0:1], pso[64:65, :, :])
                    nc.vector.scalar_tensor_tensor(out=inv[0:1], in0=tt[0:1],
                                                   scalar=2.0, in1=sd[0:1],
                                                   op0=ALU.subtract, op1=ALU.mult)
                    # broadcast inv across partitions on GpSimd (idle engine),
                    # then scale the unnormalized attn straight out of PSUM
                    bc = ivp.tile([64, 2, CH], f16, tag="bc", bufs=2)
                    nc.gpsimd.partition_broadcast(bc[:, :, :], inv[0:1, :, :])
                    # defer the normalization muls: issued now they would sit
                    # at the head of the DVE queue waiting on the GpSimd
                    # broadcast, stalling the next chunk's exp work
                    norm_pend.append((pso, bc, pr, cs))
                flush_norm()
                # Phase C for this sq-chunk (PSUM shared with the B-score tag)
                for et in range(8):
                    po = ps_b.tile([P, CH], f32, tag="b", bufs=2)
                    for t in range(4):
                        nc.tensor.matmul(po[:], lhsT=wo_t[:, t, et * P:(et + 1) * P],
                                         rhs=ou[:, t, cs], start=(t == 0), stop=(t == 3))
                    out_t = otp.tile([P, CH], f16, tag="ot", bufs=3)
                    if et % 2 == 0:
                        nc.scalar.add(out_t[:], po[:], boc[:, et:et + 1])
                    else:
                        nc.vector.tensor_scalar_add(out_t[:], po[:], boc[:, et:et + 1])
                    nc.sync.dma_start(out=yT[et * P:(et + 1) * P, cs], in_=out_t[:])

    nc.finalize()
    return nc


def _get_nc():
    if "nc" not in _cached:
        _cached["nc"] = _build()
    return _cached["nc"]


def _in_maps(query, key, value, Wq, bq, Wk, bk, Wv, bv, Wo, bo):
    query = np.asarray(query, np.float32)
    key = np.asarray(key, np.float32)
    value = np.asarray(value, np.float32)
    maps = []
    xT = {}
    for b in range(B):
        xT[("q", b)] = np.ascontiguousarray(query[b].T.astype(np.float16))
        xT[("k", b)] = np.ascontiguousarray(key[b].T.astype(np.float16))
        xT[("v", b)] = np.ascontiguousarray(value[b].T.astype(np.float16))
    for c in range(N_CORES):
        b, hh = divmod(c, 2)
        sl = slice(hh * HH, (hh + 1) * HH)

        def wcols(W):
            Ws = np.asarray(W, np.float32)[:, sl].astype(np.float16)
            return np.ascontiguousarray(Ws.reshape(8, P, HH).transpose(1, 0, 2))

        # negated: the kernel's normalization produces -attn (3-op Newton
        # yields -1/den), so -Wo restores the sign in the output projection
        wo_s = (-np.asarray(Wo, np.float32)[sl, :]).astype(np.float16)   # [512, E]
        wo_r = np.ascontiguousarray(wo_s.reshape(4, P, E).transpose(1, 0, 2))
        bo_c = (np.asarray(bo, np.float32).reshape(8, P).T if hh == 0
                else np.zeros((P, 8), np.float32))
        bv_b = np.ascontiguousarray(
            np.tile(np.asarray(bv, np.float32)[sl].astype(np.float16),
                    (P, 1)).reshape(P, 8, D))
        maps.append({
            "xqT": xT[("q", b)],
            "xkT": xT[("k", b)],
            "xvT": xT[("v", b)],
            "wq": wcols(Wq),
            "wk": wcols(Wk),
            "wv": wcols(Wv),
            "bq_col": np.ascontiguousarray(np.asarray(bq, np.float32)[sl].reshape(4, P).T),
            "bk_col": np.ascontiguousarray(np.asarray(bk, np.float32)[sl].reshape(4, P).T),
            "bv_bc": bv_b,
            "wo": wo_r,
            "bo_col": np.ascontiguousarray(bo_c),
        })
    return maps


def _assemble(results):
    outs = [results[c]["yT"] for c in range(N_CORES)]
    return np.stack([
        (outs[2 * b].astype(np.float32) + outs[2 * b + 1].astype(np.float32)).T
        for b in range(B)
    ]).astype(np.float32)


def kernel(**inputs):
    nc = _get_nc()
    maps = _in_maps(**inputs)
    r = run_bass_kernel_spmd(nc, maps, list(range(N_CORES)))
    return _assemble(r.results)


def _ensure_ntff_hook():
    """Register the axon NTFF profiling hook (missing antenv.axon_hooks shim)."""
    import contextlib
    import ctypes
    import types

    try:
        from antenv.axon_hooks import get_axon_ntff_profile_hook
        if get_axon_ntff_profile_hook() is not None:
            return
    except ImportError:
        pass

    import antenv

    holder = {}
    mod = types.ModuleType("antenv.axon_hooks")
    mod.set_axon_ntff_profile_hook = lambda h: holder.__setitem__("h", h)
    mod.get_axon_ntff_profile_hook = lambda: holder.get("h")
    sys.modules["antenv.axon_hooks"] = mod
    antenv.axon_hooks = mod

    so_path = "/opt/axon/libaxon_pjrt.so"
    lib = ctypes.CDLL(so_path)
    if not hasattr(lib, "axon_start_nrt_profile"):
        return
    lib.axon_start_nrt_profile.argtypes = [ctypes.POINTER(ctypes.c_int64), ctypes.c_size_t]
    lib.axon_start_nrt_profile.restype = ctypes.c_int64
    lib.axon_stop_nrt_profile.argtypes = [ctypes.c_char_p]
    lib.axon_stop_nrt_profile.restype = ctypes.c_int64

    @contextlib.contextmanager
    def _hook(output_dir, device_ids):
        import jax

        jax.devices()
        if device_ids:
            ids = (ctypes.c_int64 * len(device_ids))(*device_ids)
            rc = lib.axon_start_nrt_profile(ids, len(device_ids))
        else:
            rc = lib.axon_start_nrt_profile(None, 0)
        if rc != 0:
            raise RuntimeError(f"axon_start_nrt_profile rc={rc}")
        try:
            yield
        finally:
            n = lib.axon_stop_nrt_profile(str(output_dir).encode())
            if n < 0:
                raise RuntimeError(f"axon_stop_nrt_profile rc={n}")

    mod.set_axon_ntff_profile_hook(_hook)


def kernel_traced(tmpdir=None, **inputs):
    """Like kernel() but with NTFF tracing; returns (output, exec_time_ns)."""
    _ensure_ntff_hook()
    import concourse.bass_utils as bu
    bu.upload_artifacts = lambda d: d  # no artifact bucket in this container
    nc = _get_nc()
    maps = _in_maps(**inputs)
    r = run_bass_kernel_spmd(nc, maps, list(range(N_CORES)), trace=True, tmpdir=tmpdir)
    return _assemble(r.results), r.exec_time_ns


# revision 28
# speedup vs baseline: 1.0389x; 1.0389x over previous
"""Multihead attention (B=4, S=2048, E=1024, H=16, D=64) on 8 Trainium2 cores.

Sharding: core c = (batch b = c//2, head-half hh = c%2). Each core computes one
batch's attention for 8 heads (512 of the 1024 projection columns), producing a
partial output (row-split Wo); the host sums the two partials per batch.

v2 design (vs the 775us v1):
- Host pre-transposes x (xT [E,S] f16) so Phase A is pure projection matmuls
  (no PE transposes, no DVE transpose copies). All DMA'd operands are f16.
- Score matmuls for the two heads of a pair use PE row-tiling (K=64 at
  partitions 0-63 / 64-127) and run concurrently.
- exp() is split between ScalarE (true exp) and VectorE (Schraudolph bit-trick
  exp: one affine f32->int16 op whose result bitcast as f16 approximates
  exp to ~3%) so the 33M-element softmax doesn't serialize behind ScalarE.
- Phase C (output projection) is interleaved per sq-chunk with Phase B to
  keep the PE dense; output DMA'd as f16, host does the final cast/sum.
"""
import sys

sys.path.insert(0, "/opt/trn_rl_repo")

import numpy as np

import concourse.bacc as bacc
import concourse.mybir as mybir
import concourse.tile as tile
from concourse.bass_utils import run_bass_kernel_spmd

E = 1024
H = 16
D = 64
B = 4
S = 2048
HH = E // 2          # projection cols per core
N_CORES = 8
P = 128
NCH = 4              # sq-chunks of 512
CH = 512
f32 = mybir.dt.float32
f16 = mybir.dt.float16
i16 = mybir.dt.int16
AF = mybir.ActivationFunctionType
ALU = mybir.AluOpType

# Schraudolph fast exp on DVE: exp(s*0.125) ~= bitcast_f16(int16(s*SCH_A + SCH_B))
SCH_A = 0.125 * 1024.0 / float(np.log(2.0))   # 184.664
SCH_B = 15360.0 - 44.0
# fp32 bit-trick reciprocal seed (refined by one Newton step on DVE)
RMAGIC = 0x7EF311C3

_cached = {}


def _build():
    nc = bacc.Bacc(None, target_bir_lowering=False)

    xqT = nc.declare_dram_parameter("xqT", [E, S], f16, isOutput=False)
    xkT = nc.declare_dram_parameter("xkT", [E, S], f16, isOutput=False)
    xvT = nc.declare_dram_parameter("xvT", [E, S], f16, isOutput=False)
    wq = nc.declare_dram_parameter("wq", [P, 8, HH], f16, isOutput=False)
    wk = nc.declare_dram_parameter("wk", [P, 8, HH], f16, isOutput=False)
    wv = nc.declare_dram_parameter("wv", [P, 8, HH], f16, isOutput=False)
    bq_col = nc.declare_dram_parameter("bq_col", [P, 4], f32, isOutput=False)
    bk_col = nc.declare_dram_parameter("bk_col", [P, 4], f32, isOutput=False)
    bv_bc = nc.declare_dram_parameter("bv_bc", [P, 8, D], f16, isOutput=False)
    wo = nc.declare_dram_parameter("wo", [P, 4, E], f16, isOutput=False)
    bo_col = nc.declare_dram_parameter("bo_col", [P, 8], f32, isOutput=False)
    yT = nc.declare_dram_parameter("yT", [E, S], f16, isOutput=True)

    from contextlib import ExitStack

    with tile.TileContext(nc) as tc, ExitStack() as stack:
        main = stack.enter_context(tc.tile_pool(name="main", bufs=1))
        qT = main.tile([P, 4, S], f16)      # [d-in-pair, pair, sq]
        kT = main.tile([P, 4, S], f16)
        vbuf = main.tile([P, 16, 8, D + 1], f16)  # [sv, s-tile, head, d|1]
        ou = main.tile([P, 4, S], f16)      # attention out (normalized), [d-in-pair, pair, sq]
        wo_t = main.tile([P, 4, E], f16)
        bqc = main.tile([P, 4], f32)
        bkc = main.tile([P, 4], f32)
        boc = main.tile([P, 8], f32)
        bvt = main.tile([P, 8, D], f16)
        pones = main.tile([P, 64], f16)

        nc.vector.memset(pones[:], 1.0)
        nc.vector.memset(vbuf[:, :, :, D], 1.0)

        nc.sync.dma_start(out=bqc[:], in_=bq_col[:])
        nc.sync.dma_start(out=bkc[:], in_=bk_col[:])
        nc.sync.dma_start(out=bvt[:], in_=bv_bc[:])

        # ---------------- Phase A: projections (x comes in pre-transposed) ----
        with tc.tile_pool(name="wp", bufs=2) as wp, \
             tc.tile_pool(name="xp", bufs=2) as xp, \
             tc.tile_pool(name="ps_pj", bufs=4, space="PSUM") as ps_pj:
            for xdram, wdram, kind in ((xkT, wk, "k"), (xqT, wq, "q"), (xvT, wv, "v")):
                w_t = wp.tile([P, 8, HH], f16, tag="w", name=f"w_{kind}")
                nc.sync.dma_start(out=w_t[:], in_=wdram[:])
                x_t = xp.tile([P, 8, S], f16, tag="x", name=f"x_{kind}")
                for kc in range(8):
                    nc.gpsimd.dma_start(out=x_t[:, kc, :],
                                        in_=xdram[kc * P:(kc + 1) * P, :])
                if kind == "v":
                    for sv in range(16):
                        pp = ps_pj.tile([P, 8, D], f32, tag="pj", name=f"pj_v{sv}")
                        for kc in range(8):
                            nc.tensor.matmul(pp[:], lhsT=x_t[:, kc, sv * P:(sv + 1) * P],
                                             rhs=w_t[:, kc, :],
                                             start=(kc == 0), stop=(kc == 7))
                        nc.vector.tensor_add(vbuf[:, sv, :, 0:D], pp[:], bvt[:])
                else:
                    dest = qT if kind == "q" else kT
                    bcol = bqc if kind == "q" else bkc
                    for u in range(4):
                        for g in range(4):
                            pp = ps_pj.tile([P, CH], f32, tag="pj",
                                            name=f"pj_{kind}{u}{g}")
                            for kc in range(8):
                                nc.tensor.matmul(pp[:], lhsT=w_t[:, kc, u * P:(u + 1) * P],
                                                 rhs=x_t[:, kc, g * CH:(g + 1) * CH],
                                                 start=(kc == 0), stop=(kc == 7))
                            nc.scalar.add(dest[:, u, g * CH:(g + 1) * CH],
                                          pp[:], bcol[:, u:u + 1])

        nc.sync.dma_start(out=boc[:], in_=bo_col[:])
        nc.sync.dma_start(out=wo_t[:], in_=wo[:])

        # ---------------- Phase B: attention, Phase C: out-proj (interleaved per c)
        with tc.tile_pool(name="ep", bufs=1) as ep, \
             tc.tile_pool(name="ivp", bufs=2) as ivp, \
             tc.tile_pool(name="otp", bufs=3) as otp, \
             tc.tile_pool(name="ps_a", bufs=2, space="PSUM") as ps_a, \
             tc.tile_pool(name="ps_b", bufs=2, space="PSUM") as ps_b, \
             tc.tile_pool(name="ps_ac", bufs=2, space="PSUM") as ps_ac:
            den_pend = []
            norm_pend = []

            def flush_den():
                # -1/den: bit-trick seed + one Newton step on DVE (sign folded
                # into Wo on the host), then GpSimd broadcasts across lanes
                while den_pend:
                    fpso, fpr, fcs = den_pend.pop(0)
                    sd = ivp.tile([1, 2, CH], f32, tag="sd", bufs=2)
                    tt = ivp.tile([1, 2, CH], f32, tag="tt", bufs=2)
                    inv = ivp.tile([1, 2, CH], f16, tag="iv", bufs=2)
                    nc.vector.tensor_scalar(out=sd[0:1].bitcast(mybir.dt.int32),
                                            in0=fpso[64:65, :, :].bitcast(mybir.dt.int32),
                                            scalar1=-1, scalar2=RMAGIC,
                                            op0=ALU.mult, op1=ALU.add)
                    nc.vector.tensor_mul(tt[0:1], sd[0:1], fpso[64:65, :, :])
                    nc.vector.scalar_tensor_tensor(out=inv[0:1], in0=tt[0:1],
                                                   scalar=2.0, in1=sd[0:1],
                                                   op0=ALU.subtract, op1=ALU.mult)
                    bc = ivp.tile([64, 2, CH], f16, tag="bc", bufs=2)
                    nc.gpsimd.partition_broadcast(bc[:, :, :], inv[0:1, :, :])
                    norm_pend.append((fpso, bc, fpr, fcs))

            def flush_norm():
                while norm_pend:
                    fpso, fbc, fpr, fcs = norm_pend.pop(0)
                    nc.vector.tensor_mul(ou[0:64, fpr, fcs], fpso[0:64, 0, :],
                                         fbc[:, 0, :])
                    nc.vector.tensor_mul(ou[64:128, fpr, fcs], fpso[0:64, 1, :],
                                         fbc[:, 1, :])

            for c in range(NCH):
                cs = slice(c * CH, (c + 1) * CH)
                for pr in range(4):
                    hA, hB = 2 * pr, 2 * pr + 1
                    # A and B halves share one 2-bank accumulator so the two
                    # softmax denominators form a single [1, 1024] row
                    pso = ps_ac.tile([D + 1, 2, CH], f32, tag="p", bufs=2)
                    pend = []
                    for st in range(16):
                        ks = slice(st * P, (st + 1) * P)
                        pscA = ps_a.tile([P, CH], f32, tag="a", bufs=2)
                        pscB = ps_b.tile([P, CH], f32, tag="b", bufs=2)
                        # two concurrent K=64 row-tiled score matmuls
                        nc.tensor.matmul(pscA[:], lhsT=kT[0:64, pr, ks],
                                         rhs=qT[0:64, pr, cs], start=True, stop=True)
                        nc.tensor.matmul(pscB[:], lhsT=kT[64:128, pr, ks],
                                         rhs=qT[64:128, pr, cs], start=True, stop=True)
                        exA = ep.tile([P, CH], f16, tag="xa", bufs=6)
                        exB = ep.tile([P, CH], f16, tag="xb", bufs=6)
                        # exp split (~20 ScalarE / 12 DVE per chunk): ScalarE
                        # true exp, DVE Schraudolph bit-trick exp
                        nc.scalar.activation(exA[:], pscA[:], AF.Exp, scale=0.125)
                        if st % 4 == 0:
                            nc.scalar.activation(exB[:], pscB[:], AF.Exp, scale=0.125)
                        else:
                            nc.vector.tensor_scalar(out=exB[:].bitcast(i16),
                                                    in0=pscB[:], scalar1=SCH_A,
                                                    scalar2=SCH_B,
                                                    op0=ALU.mult, op1=ALU.add)
                        # attnV runs two steps behind scores on the PE queue:
                        # by the time the PE reaches attnV(st-2), exp(st-2) is
                        # long done, so the PE never stalls mid-chunk (stalls
                        # break the HAM activity window and halve the clock)
                        if st == 3:
                            flush_den()
                        elif st == 8:
                            flush_norm()
                        pend.append((st, exA, exB))
                        if len(pend) > 3:
                            pst, pA, pB = pend.pop(0)
                            nc.tensor.matmul(pso[:, 0, :], lhsT=vbuf[:, pst, hA, :],
                                             rhs=pA[:], start=(pst == 0), stop=False,
                                             skip_group_check=True)
                            nc.tensor.matmul(pso[:, 1, :], lhsT=vbuf[:, pst, hB, :],
                                             rhs=pB[:], start=(pst == 0), stop=False,
                                             skip_group_check=True)
                    for pst, pA, pB in pend:
                        nc.tensor.matmul(pso[:, 0, :], lhsT=vbuf[:, pst, hA, :],
                                         rhs=pA[:], start=False, stop=(pst == 15),
                                         skip_group_check=True)
                        nc.tensor.matmul(pso[:, 1, :], lhsT=vbuf[:, pst, hB, :],
                                         rhs=pB[:], start=False, stop=(pst == 15),
                                         skip_group_check=True)
                    # defer the whole den/normalization block into the next
                    # chunk's stream: emitted here, the serial Newton chain
                    # plus the broadcast wait would clog the DVE FIFO right
                    # when the next chunk's exp work needs it
                    den_pend.append((pso, pr, cs))
                flush_den()
                flush_norm()
                # Phase C for this sq-chunk (PSUM shared with the B-score tag)
                for et in range(8):
                    po = ps_b.tile([P, CH], f32, tag="b", bufs=2)
                    for t in range(4):
                        nc.tensor.matmul(po[:], lhsT=wo_t[:, t, et * P:(et + 1) * P],
                                         rhs=ou[:, t, cs], start=(t == 0), stop=(t == 3))
                    out_t = otp.tile([P, CH], f16, tag="ot", bufs=3)
                    if et % 2 == 0:
                        nc.scalar.add(out_t[:], po[:], boc[:, et:et + 1])
                    else:
                        nc.vector.tensor_scalar_add(out_t[:], po[:], boc[:, et:et + 1])
                    nc.sync.dma_start(out=yT[et * P:(et + 1) * P, cs], in_=out_t[:])

    nc.finalize()
    return nc


def _get_nc():
    if "nc" not in _cached:
        _cached["nc"] = _build()
    return _cached["nc"]


def _in_maps(query, key, value, Wq, bq, Wk, bk, Wv, bv, Wo, bo):
    query = np.asarray(query, np.float32)
    key = np.asarray(key, np.float32)
    value = np.asarray(value, np.float32)
    maps = []
    xT = {}
    for b in range(B):
        xT[("q", b)] = np.ascontiguousarray(query[b].T.astype(np.float16))
        xT[("k", b)] = np.ascontiguousarray(key[b].T.astype(np.float16))
        xT[("v", b)] = np.ascontiguousarray(value[b].T.astype(np.float16))
    for c in range(N_CORES):
        b, hh = divmod(c, 2)
        sl = slice(hh * HH, (hh + 1) * HH)

        def wcols(W):
            Ws = np.asarray(W, np.float32)[:, sl].astype(np.float16)
            return np.ascontiguousarray(Ws.reshape(8, P, HH).transpose(1, 0, 2))

        # negated: the kernel's normalization produces -attn (3-op Newton
        # yields -1/den), so -Wo restores the sign in the output projection
        wo_s = (-np.asarray(Wo, np.float32)[sl, :]).astype(np.float16)   # [512, E]
        wo_r = np.ascontiguousarray(wo_s.reshape(4, P, E).transpose(1, 0, 2))
        bo_c = (np.asarray(bo, np.float32).reshape(8, P).T if hh == 0
                else np.zeros((P, 8), np.float32))
        bv_b = np.ascontiguousarray(
            np.tile(np.asarray(bv, np.float32)[sl].astype(np.float16),
                    (P, 1)).reshape(P, 8, D))
        maps.append({
            "xqT": xT[("q", b)],
            "xkT": xT[("k", b)],
            "xvT": xT[("v", b)],
            "wq": wcols(Wq),
            "wk": wcols(Wk),
            "wv": wcols(Wv),
            "bq_col": np.ascontiguousarray(np.asarray(bq, np.float32)[sl].reshape(4, P).T),
            "bk_col": np.ascontiguousarray(np.asarray(bk, np.float32)[sl].reshape(4, P).T),
            "bv_bc": bv_b,
            "wo": wo_r,
            "bo_col": np.ascontiguousarray(bo_c),
        })
    return maps


def _assemble(results):
    outs = [results[c]["yT"] for c in range(N_CORES)]
    return np.stack([
        (outs[2 * b].astype(np.float32) + outs[2 * b + 1].astype(np.float32)).T
        for b in range(B)
    ]).astype(np.float32)


def kernel(**inputs):
    nc = _get_nc()
    maps = _in_maps(**inputs)
    r = run_bass_kernel_spmd(nc, maps, list(range(N_CORES)))
    return _assemble(r.results)


def _ensure_ntff_hook():
    """Register the axon NTFF profiling hook (missing antenv.axon_hooks shim)."""
    import contextlib
    import ctypes
    import types

    try:
        from antenv.axon_hooks import get_axon_ntff_profile_hook
        if get_axon_ntff_profile_hook() is not None:
            return
    except ImportError:
        pass

    import antenv

    holder = {}
    mod = types.ModuleType("antenv.axon_hooks")
    mod.set_axon_ntff_profile_hook = lambda h: holder.__setitem__("h", h)
    mod.get_axon_ntff_profile_hook = lambda: holder.get("h")
    sys.modules["antenv.axon_hooks"] = mod
    antenv.axon_hooks = mod

    so_path = "/opt/axon/libaxon_pjrt.so"
    lib = ctypes.CDLL(so_path)
    if not hasattr(lib, "axon_start_nrt_profile"):
        return
    lib.axon_start_nrt_profile.argtypes = [ctypes.POINTER(ctypes.c_int64), ctypes.c_size_t]
    lib.axon_start_nrt_profile.restype = ctypes.c_int64
    lib.axon_stop_nrt_profile.argtypes = [ctypes.c_char_p]
    lib.axon_stop_nrt_profile.restype = ctypes.c_int64

    @contextlib.contextmanager
    def _hook(output_dir, device_ids):
        import jax

        jax.devices()
        if device_ids:
            ids = (ctypes.c_int64 * len(device_ids))(*device_ids)
            rc = lib.axon_start_nrt_profile(ids, len(device_ids))
        else:
            rc = lib.axon_start_nrt_profile(None, 0)
        if rc != 0:
            raise RuntimeError(f"axon_start_nrt_profile rc={rc}")
        try:
            yield
        finally:
            n = lib.axon_stop_nrt_profile(str(output_dir).encode())
            if n < 0:
                raise RuntimeError(f"axon_stop_nrt_profile rc={n}")

    mod.set_axon_ntff_profile_hook(_hook)


def kernel_traced(tmpdir=None, **inputs):
    """Like kernel() but with NTFF tracing; returns (output, exec_time_ns)."""
    _ensure_ntff_hook()
    import concourse.bass_utils as bu
    bu.upload_artifacts = lambda d: d  # no artifact bucket in this container
    nc = _get_nc()
    maps = _in_maps(**inputs)
    r = run_bass_kernel_spmd(nc, maps, list(range(N_CORES)), trace=True, tmpdir=tmpdir)
    return _assemble(r.results), r.exec_time_ns
